# revision 2
# baseline (speedup 1.0000x reference)
"""Trainium2 Bass kernel for nn_Block_47261820125190 (dense transformer block).

Contract: kernel(**inputs) takes FULL inputs (x [8192,16,256] + weights),
shards batch across 8 NeuronCores (data parallel), runs a fused Bass/Tile
kernel per core, returns FULL output [8192,16,256] fp32.
"""

import sys

for p in ("/opt/trn_rl_repo",):
    if p not in sys.path:
        sys.path.insert(0, p)

import numpy as np
import ml_dtypes
import os as _os
SKIP_ATTN = _os.environ.get('SKIP_ATTN','0')=='1'
SKIP_MLP = _os.environ.get('SKIP_MLP','0')=='1'
SKIP_SMAX = _os.environ.get('SKIP_SMAX','0')=='1'
from contextlib import ExitStack

import concourse.bass as bass
import concourse.tile as tile
from concourse import bacc
from concourse import mybir
from concourse.bass_utils import run_bass_kernel_spmd

# Model dims (hardcoded per spec)
B, T, C, H = 8192, 16, 256, 4
HS = C // H          # 64
FF = 4 * C           # 1024
EPS = 1e-5
NCORES = 8
NTOK = (B // NCORES) * T   # 16384 tokens per core
TT = 512                   # tokens per big tile
NST = TT // 128            # 4 subtiles of 128 tokens
NTILES = NTOK // TT        # 32
CINV = float(C) ** -0.5

F32 = mybir.dt.float32
F32R = mybir.dt.float32r
BF16 = mybir.dt.bfloat16


def _r(ap):
    return ap.bitcast(F32R)


def build_kernel():
    nc = bacc.Bacc(None)

    x_d = nc.declare_dram_parameter("x", [NTOK, C], F32, isOutput=False)
    wqkv_d = nc.declare_dram_parameter("wqkv", [C, 3 * C], F32R, isOutput=False)
    bqk_d = nc.declare_dram_parameter("bqk", [128, 4], F32, isOutput=False)
    bv_d = nc.declare_dram_parameter("bv", [C], F32, isOutput=False)
    wo_d = nc.declare_dram_parameter("wo", [C, C], F32R, isOutput=False)
    bo_d = nc.declare_dram_parameter("bo", [C], F32, isOutput=False)
    w1_d = nc.declare_dram_parameter("w1", [C, FF], F32R, isOutput=False)
    b1m_d = nc.declare_dram_parameter("b1m", [128, FF // 128], F32, isOutput=False)
    w2_d = nc.declare_dram_parameter("w2", [FF, C], F32R, isOutput=False)
    b2_d = nc.declare_dram_parameter("b2", [C], F32, isOutput=False)
    mask_d = nc.declare_dram_parameter("maskb", [128, 128], F32, isOutput=False)
    id_d = nc.declare_dram_parameter("ident", [128, 128], F32R, isOutput=False)
    idb_d = nc.declare_dram_parameter("identb", [128, 128], BF16, isOutput=False)
    out_d = nc.declare_dram_parameter("out", [NTOK, C], F32, isOutput=True)

    def bcast(ap_1d, n):
        # view a [n] dram vector as [128, n] with 0-step partition dim
        return bass.AP(tensor=ap_1d.tensor, offset=ap_1d.offset,
                       ap=[[0, 128]] + list(ap_1d.ap))

    with TileCtx(nc) as tc:
        ctx = tc._ctx
        singles = ctx.enter_context(tc.tile_pool(name="singles", bufs=1))
        acts = ctx.enter_context(tc.tile_pool(name="acts", bufs=2))
        small = ctx.enter_context(tc.tile_pool(name="small", bufs=8))
        attnp = ctx.enter_context(tc.tile_pool(name="attnp", bufs=3))
        psA = ctx.enter_context(tc.tile_pool(name="psA", bufs=3, space="PSUM"))
        psB = ctx.enter_context(tc.tile_pool(name="psB", bufs=2, space="PSUM"))
        psC = ctx.enter_context(tc.tile_pool(name="psC", bufs=3, space="PSUM"))

        # ---- persistent weights/constants in SBUF ----
        wqkv_sb = []
        for k in range(2):
            t_ = singles.tile([128, 3 * C], F32R, tag=f"wqkv{k}")
            nc.sync.dma_start(out=t_, in_=wqkv_d[k * 128:(k + 1) * 128, :])
            wqkv_sb.append(t_)
        wo_sb = []
        for k in range(2):
            t_ = singles.tile([128, C], F32R, tag=f"wo{k}")
            nc.sync.dma_start(out=t_, in_=wo_d[k * 128:(k + 1) * 128, :])
            wo_sb.append(t_)
        w1_sb = []
        for k in range(2):
            t_ = singles.tile([128, FF], F32R, tag=f"w1{k}")
            nc.sync.dma_start(out=t_, in_=w1_d[k * 128:(k + 1) * 128, :])
            w1_sb.append(t_)
        w2_sb = []
        for k in range(8):
            t_ = singles.tile([128, C], F32R, tag=f"w2{k}")
            nc.sync.dma_start(out=t_, in_=w2_d[k * 128:(k + 1) * 128, :])
            w2_sb.append(t_)
        bqk_sb = singles.tile([128, 4], F32, tag="bqk")
        nc.sync.dma_start(out=bqk_sb, in_=bqk_d[:, :])
        b1m_sb = singles.tile([128, FF // 128], F32, tag="b1m")
        nc.sync.dma_start(out=b1m_sb, in_=b1m_d[:, :])
        bv_b = singles.tile([128, C], F32, tag="bv")
        nc.sync.dma_start(out=bv_b, in_=bcast(bv_d[:], C))
        bo_b = singles.tile([128, C], F32, tag="bo")
        nc.sync.dma_start(out=bo_b, in_=bcast(bo_d[:], C))
        b2_b = singles.tile([128, C], F32, tag="b2")
        nc.sync.dma_start(out=b2_b, in_=bcast(b2_d[:], C))
        mask_sb = singles.tile([128, 128], F32, tag="mask")
        nc.sync.dma_start(out=mask_sb, in_=mask_d[:, :])
        idf_sb = singles.tile([128, 128], F32R, tag="idf")
        nc.sync.dma_start(out=idf_sb, in_=id_d[:, :])
        idb_sb = singles.tile([128, 128], BF16, tag="idb")
        nc.sync.dma_start(out=idb_sb, in_=idb_d[:, :])
        eps_sb = singles.tile([128, 1], F32, tag="eps")
        nc.vector.memset(eps_sb, EPS)

        def layernorm(x_sb, h_sb, st):
            stats = small.tile([128, 6], F32, tag="stats")
            mv = small.tile([128, 2], F32, tag="mv")
            rstd = small.tile([128, 1], F32, tag="rstd")
            nc.vector.bn_stats(out=stats, in_=x_sb)
            nc.vector.bn_aggr(out=mv, in_=stats)
            nc.scalar.activation(out=rstd, in_=mv[:, 1:2],
                                 func=mybir.ActivationFunctionType.Sqrt,
                                 bias=eps_sb, scale=1.0)
            nc.vector.reciprocal(out=rstd, in_=rstd)
            nc.vector.tensor_scalar(out=h_sb, in0=x_sb,
                                    scalar1=mv[:, 0:1], scalar2=rstd,
                                    op0=mybir.AluOpType.subtract,
                                    op1=mybir.AluOpType.mult)

        for it in range(NTILES):
            base = (it % (NTOK // TT)) * TT
            # ---- load x, LN1, transpose h ----
            x_sb = acts.tile([128, NST, C], F32, tag="x")
            h_sb = acts.tile([128, NST, C], F32R, tag="h")
            hT_sb = [acts.tile([128, TT], F32R, tag=f"hT{k}", name=f"hT{k}") for k in range(2)]
            for st in range(NST):
                nc.sync.dma_start(
                    out=x_sb[:, st, :],
                    in_=x_d[base + st * 128: base + (st + 1) * 128, :])
                layernorm(x_sb[:, st, :], h_sb[:, st, :], st)
                for cc in range(2):
                    tp = psB.tile([128, 128], F32, tag="tp")
                    nc.tensor.transpose(out=_r(tp), in_=_r(h_sb[:, st, cc * 128:(cc + 1) * 128]),
                                        identity=_r(idf_sb))
                    nc.vector.tensor_copy(out=hT_sb[cc][:, st * 128:(st + 1) * 128], in_=tp)

            # ---- QKV ----
            qT_sb = [attnp.tile([128, TT], BF16, tag=f"qT{m}", name=f"qT{m}") for m in range(2)]
            kT_sb = [attnp.tile([128, TT], BF16, tag=f"kT{m}", name=f"kT{m}") for m in range(2)]
            for m in range(4):  # 0,1 -> q chunks; 2,3 -> k chunks
                ps = psA.tile([128, TT], F32, tag="psA")
                for k in range(2):
                    nc.tensor.matmul(out=ps,
                                     lhsT=_r(wqkv_sb[k][:, m * 128:(m + 1) * 128]),
                                     rhs=_r(hT_sb[k]),
                                     start=(k == 0), stop=(k == 1))
                dst = qT_sb[m] if m < 2 else kT_sb[m - 2]
                nc.vector.tensor_scalar(out=dst, in0=ps,
                                        scalar1=bqk_sb[:, m:m + 1],
                                        scalar2=CINV if m < 2 else 1.0,
                                        op0=mybir.AluOpType.add,
                                        op1=mybir.AluOpType.mult)
            v_sb = attnp.tile([128, NST, C], BF16, tag="v")
            for st in range(NST):
                ps = psC.tile([128, C], F32, tag="psC")
                for k in range(2):
                    nc.tensor.matmul(out=ps,
                                     lhsT=_r(hT_sb[k][:, st * 128:(st + 1) * 128]),
                                     rhs=_r(wqkv_sb[k][:, 2 * C:3 * C]),
                                     start=(k == 0), stop=(k == 1))
                nc.vector.tensor_add(out=v_sb[:, st, :], in0=ps, in1=bv_b)

            # ---- attention ----
            attnT_sb = [acts.tile([128, TT], F32R, tag=f"aT{hc}", name=f"aT{hc}") for hc in range(2)]
            attnT_ps = [psA.tile([128, TT], F32, tag="psA", name=f"aTps{hc}") for hc in range(2)]
            if SKIP_ATTN:
                for hc in range(2):
                    nc.vector.tensor_copy(out=attnT_sb[hc], in_=hT_sb[hc])
            for st in range(NST if not SKIP_ATTN else 0):
                for h in range(H):
                    hc, off = h // 2, 64 * (h % 2)
                    sl = slice(st * 128, (st + 1) * 128)
                    s_ps = psC.tile([128, 128], F32, tag="psC")
                    nc.tensor.matmul(out=s_ps,
                                     lhsT=qT_sb[hc][off:off + 64, sl],
                                     rhs=kT_sb[hc][off:off + 64, sl],
                                     start=True, stop=True)
                    wn_sb = small.tile([128, 128], BF16, tag="wn")
                    if SKIP_SMAX:
                        nc.vector.tensor_copy(out=wn_sb, in_=s_ps)
                    else:
                        nc.vector.tensor_add(out=s_ps, in0=s_ps, in1=mask_sb)
                        w_sb = small.tile([128, 128], BF16, tag="w")
                        rsum = small.tile([128, 1], F32, tag="rsum")
                        nc.scalar.activation(out=w_sb, in_=s_ps,
                                             func=mybir.ActivationFunctionType.Exp,
                                             accum_out=rsum)
                        rcp = small.tile([128, 1], F32, tag="rcp")
                        nc.vector.reciprocal(out=rcp, in_=rsum)
                        nc.gpsimd.tensor_scalar_mul(out=wn_sb, in0=w_sb, scalar1=rcp)
                    wt_ps = psB.tile([128, 128], BF16, tag="tp")
                    nc.tensor.transpose(out=wt_ps, in_=wn_sb, identity=idb_sb)
                    wt_sb = small.tile([128, 128], BF16, tag="wt")
                    nc.vector.tensor_copy(out=wt_sb, in_=wt_ps)
                    nc.tensor.matmul(out=attnT_ps[hc][off:off + 64, sl],
                                     lhsT=v_sb[:, st, h * 64:(h + 1) * 64],
                                     rhs=wt_sb,
                                     start=True, stop=True)
            for hc in range(2 if not SKIP_ATTN else 0):
                nc.vector.tensor_copy(out=attnT_sb[hc], in_=attnT_ps[hc])

            # ---- Wo + residual, LN2, transpose h2 ----
            x2_sb = acts.tile([128, NST, C], F32, tag="x2")
            h2_sb = acts.tile([128, NST, C], F32R, tag="h2")
            h2T_sb = [acts.tile([128, TT], F32R, tag=f"h2T{k}", name=f"h2T{k}") for k in range(2)]
            for st in range(NST):
                ps = psC.tile([128, C], F32, tag="psC")
                for hc in range(2):
                    nc.tensor.matmul(out=ps,
                                     lhsT=_r(attnT_sb[hc][:, st * 128:(st + 1) * 128]),
                                     rhs=_r(wo_sb[hc]),
                                     start=(hc == 0), stop=(hc == 1))
                nc.vector.scalar_tensor_tensor(out=x2_sb[:, st, :], in0=ps,
                                               scalar=1.0, in1=x_sb[:, st, :],
                                               op0=mybir.AluOpType.mult,
                                               op1=mybir.AluOpType.add)
                nc.gpsimd.tensor_add(out=x2_sb[:, st, :], in0=x2_sb[:, st, :], in1=bo_b)
                layernorm(x2_sb[:, st, :], h2_sb[:, st, :], st)
                for cc in range(2):
                    tp = psB.tile([128, 128], F32, tag="tp")
                    nc.tensor.transpose(out=_r(tp), in_=_r(h2_sb[:, st, cc * 128:(cc + 1) * 128]),
                                        identity=_r(idf_sb))
                    nc.vector.tensor_copy(out=h2T_sb[cc][:, st * 128:(st + 1) * 128], in_=tp)

            # ---- MLP ----
            m1_sb = [acts.tile([128, TT], F32R, tag=f"m1{mf}", name=f"m1{mf}") for mf in range(8)]
            for mf in range(8 if not SKIP_MLP else 0):
                ps = psA.tile([128, TT], F32, tag="psA")
                for k in range(2):
                    nc.tensor.matmul(out=ps,
                                     lhsT=_r(w1_sb[k][:, mf * 128:(mf + 1) * 128]),
                                     rhs=_r(h2T_sb[k]),
                                     start=(k == 0), stop=(k == 1))
                nc.scalar.activation(out=m1_sb[mf], in_=ps,
                                     func=mybir.ActivationFunctionType.Relu,
                                     bias=b1m_sb[:, mf:mf + 1], scale=1.0)
            o_sb = acts.tile([128, NST, C], F32, tag="o")
            for st in range(NST):
                ps = psC.tile([128, C], F32, tag="psC")
                if SKIP_MLP:
                    nc.vector.memset(ps, 0.0)
                for mf in range(8 if not SKIP_MLP else 0):
                    nc.tensor.matmul(out=ps,
                                     lhsT=_r(m1_sb[mf][:, st * 128:(st + 1) * 128]),
                                     rhs=_r(w2_sb[mf]),
                                     start=(mf == 0), stop=(mf == 7))
                nc.vector.scalar_tensor_tensor(out=o_sb[:, st, :], in0=ps,
                                               scalar=1.0, in1=x2_sb[:, st, :],
                                               op0=mybir.AluOpType.mult,
                                               op1=mybir.AluOpType.add)
                nc.gpsimd.tensor_add(out=o_sb[:, st, :], in0=o_sb[:, st, :], in1=b2_b)
                nc.sync.dma_start(
                    out=out_d[base + st * 128: base + (st + 1) * 128, :],
                    in_=o_sb[:, st, :])
    nc.finalize()
    return nc


class TileCtx:
    """TileContext wrapper carrying an ExitStack for pools."""

    def __init__(self, nc):
        self._tc = tile.TileContext(nc)
        self._ctx = ExitStack()

    def __enter__(self):
        tc = self._tc.__enter__()
        tc._ctx = self._ctx
        return tc

    def __exit__(self, *a):
        self._ctx.close()
        return self._tc.__exit__(*a)


def _prep_weights(Wq, Wk, Wv, Wo, bo, W1, b1, W2, b2, g1, be1, g2, be2):
    f = np.float32
    Wqf = np.transpose(Wq, (1, 0, 2)).reshape(C, C)  # [c,(h,d)]
    Wkf = np.transpose(Wk, (1, 0, 2)).reshape(C, C)
    Wvf = np.transpose(Wv, (1, 0, 2)).reshape(C, C)
    g1c = g1[:, None]
    wqkv = np.concatenate([g1c * Wqf, g1c * Wkf, g1c * Wvf], axis=1).astype(f)
    bq = be1 @ Wqf
    bk = be1 @ Wkf
    bv = (be1 @ Wvf).astype(f)
    bqk = np.stack([bq[:128], bq[128:], bk[:128], bk[128:]], axis=1).astype(f)
    w1 = (g2[:, None] * W1).astype(f)
    b1e = (b1 + be2 @ W1).astype(f)
    b1m = b1e.reshape(8, 128).T.copy()  # [128, 8]
    # block-diag causal additive mask [128,128]
    m = np.full((128, 128), -1e30, dtype=f)
    for j in range(8):
        blk = np.tril(np.zeros((T, T), dtype=f) + 0.0) * 0.0
        tri = np.triu(np.full((T, T), -1e30, dtype=f), 1)
        m[j * T:(j + 1) * T, j * T:(j + 1) * T] = tri
    return dict(wqkv=wqkv, bqk=bqk, bv=bv, wo=Wo.astype(f), bo=bo.astype(f),
                w1=w1, b1m=b1m.astype(f), w2=W2.astype(f), b2=b2.astype(f),
                maskb=m, ident=np.eye(128, dtype=f),
                identb=np.eye(128).astype(ml_dtypes.bfloat16))


_CACHE = {}


def kernel(x, Wq, Wk, Wv, Wo, bo, W1, b1, W2, b2, g1, be1, g2, be2):
    x = np.asarray(x, dtype=np.float32)
    wts = _prep_weights(np.asarray(Wq), np.asarray(Wk), np.asarray(Wv),
                        np.asarray(Wo), np.asarray(bo), np.asarray(W1),
                        np.asarray(b1), np.asarray(W2), np.asarray(b2),
                        np.asarray(g1), np.asarray(be1), np.asarray(g2),
                        np.asarray(be2))
    if "nc" not in _CACHE:
        _CACHE["nc"] = build_kernel()
    nc = _CACHE["nc"]
    xs = x.reshape(NCORES, NTOK, C)
    in_maps = [dict(x=np.ascontiguousarray(xs[i]), **wts) for i in range(NCORES)]
    r = run_bass_kernel_spmd(nc, in_maps, list(range(NCORES)))
    _CACHE["last_results"] = r
    res = r.results
    out = np.stack([res[i]["out"] for i in range(NCORES)], axis=0)
    return out.reshape(B, T, C).astype(np.float32)


if __name__ == "__main__":
    nc = build_kernel()
    print("kernel traced OK")



# revision 35
# speedup vs baseline: 1.7260x; 1.7260x over previous
"""Trainium2 Bass kernel for nn_Block_47261820125190 (dense transformer block).

Contract: kernel(**inputs) takes FULL inputs (x [8192,16,256] + weights),
shards batch across 8 NeuronCores (data parallel), runs a fused Bass/Tile
kernel per core, returns FULL output [8192,16,256] fp32.
"""

import sys

for p in ("/opt/trn_rl_repo",):
    if p not in sys.path:
        sys.path.insert(0, p)

import numpy as np
import ml_dtypes
import os as _os
SKIP_ATTN = _os.environ.get('SKIP_ATTN','0')=='1'
SKIP_MLP = _os.environ.get('SKIP_MLP','0')=='1'
SKIP_SMAX = _os.environ.get('SKIP_SMAX','0')=='1'
from contextlib import ExitStack

import concourse.bass as bass
import concourse.tile as tile
from concourse import bacc
from concourse import mybir
from concourse.bass_utils import run_bass_kernel_spmd

# Model dims (hardcoded per spec)
B, T, C, H = 8192, 16, 256, 4
HS = C // H          # 64
FF = 4 * C           # 1024
EPS = 1e-5
NCORES = 8
NTOK = (B // NCORES) * T   # 16384 tokens per core
TT = 512                   # tokens per big tile
NST = TT // 128            # 4 subtiles of 128 tokens
NTILES = NTOK // TT        # 32
CINV = float(C) ** -0.5

F32 = mybir.dt.float32
F32R = mybir.dt.float32r
BF16 = mybir.dt.bfloat16


def _r(ap):
    return ap.bitcast(F32R)


WITH_BIAS = True


def build_kernel():
    nc = bacc.Bacc(None)

    x_d = nc.declare_dram_parameter("x", [NTOK, C], F32, isOutput=False)
    wqkv_d = nc.declare_dram_parameter("wqkv", [C, 3 * C], F32R, isOutput=False)
    bqk_d = nc.declare_dram_parameter("bqk", [128, 4], F32, isOutput=False)
    bv_d = nc.declare_dram_parameter("bv", [C], F32, isOutput=False)
    wo_d = nc.declare_dram_parameter("wo", [C, C], F32R, isOutput=False)
    bo_d = nc.declare_dram_parameter("bo", [C], F32, isOutput=False)
    w1_d = nc.declare_dram_parameter("w1", [C, FF], F32R, isOutput=False)
    b1m_d = nc.declare_dram_parameter("b1m", [128, FF // 128], F32, isOutput=False)
    w2_d = nc.declare_dram_parameter("w2", [FF, C], F32R, isOutput=False)
    b2_d = nc.declare_dram_parameter("b2", [C], F32, isOutput=False)
    mask_d = nc.declare_dram_parameter("maskb", [128, 128], F32, isOutput=False)
    id_d = nc.declare_dram_parameter("ident", [128, 128], F32R, isOutput=False)
    idb_d = nc.declare_dram_parameter("identb", [128, 128], BF16, isOutput=False)
    out_d = nc.declare_dram_parameter("out", [NTOK, C], F32, isOutput=True)

    def bcast(ap_1d, n):
        # view a [n] dram vector as [128, n] with 0-step partition dim
        return bass.AP(tensor=ap_1d.tensor, offset=ap_1d.offset,
                       ap=[[0, 128]] + list(ap_1d.ap))

    with TileCtx(nc) as tc:
        ctx = tc._ctx
        singles = ctx.enter_context(tc.tile_pool(name="singles", bufs=1))
        acts = ctx.enter_context(tc.tile_pool(name="acts", bufs=2))
        small = ctx.enter_context(tc.tile_pool(name="small", bufs=8))
        attnp = ctx.enter_context(tc.tile_pool(name="attnp", bufs=3))
        psA = ctx.enter_context(tc.tile_pool(name="psA", bufs=3, space="PSUM"))
        psB = ctx.enter_context(tc.tile_pool(name="psB", bufs=2, space="PSUM"))
        psC = ctx.enter_context(tc.tile_pool(name="psC", bufs=3, space="PSUM"))

        # ---- persistent weights/constants in SBUF ----
        wqkv_sb = []
        for k in range(2):
            t_ = singles.tile([128, 3 * C], F32R, tag=f"wqkv{k}")
            nc.sync.dma_start(out=t_, in_=wqkv_d[k * 128:(k + 1) * 128, :])
            wqkv_sb.append(t_)
        wo_sb = []
        for k in range(2):
            t_ = singles.tile([128, C], F32R, tag=f"wo{k}")
            nc.sync.dma_start(out=t_, in_=wo_d[k * 128:(k + 1) * 128, :])
            wo_sb.append(t_)
        w1_sb = []
        for k in range(2):
            t_ = singles.tile([128, FF], F32R, tag=f"w1{k}")
            nc.sync.dma_start(out=t_, in_=w1_d[k * 128:(k + 1) * 128, :])
            w1_sb.append(t_)
        w2_sb = []
        for k in range(8):
            t_ = singles.tile([128, C], F32R, tag=f"w2{k}")
            nc.sync.dma_start(out=t_, in_=w2_d[k * 128:(k + 1) * 128, :])
            w2_sb.append(t_)
        bqk_sb = singles.tile([128, 4], F32, tag="bqk")
        nc.sync.dma_start(out=bqk_sb, in_=bqk_d[:, :])
        b1m_sb = singles.tile([128, FF // 128], F32, tag="b1m")
        nc.sync.dma_start(out=b1m_sb, in_=b1m_d[:, :])
        bv_b = singles.tile([128, C], F32, tag="bv")
        nc.sync.dma_start(out=bv_b, in_=bcast(bv_d[:], C))
        bo_b = singles.tile([128, C], F32, tag="bo")
        nc.sync.dma_start(out=bo_b, in_=bcast(bo_d[:], C))
        b2_b = singles.tile([128, C], F32, tag="b2")
        nc.sync.dma_start(out=b2_b, in_=bcast(b2_d[:], C))
        mask_sb = singles.tile([128, 128], F32, tag="mask")
        nc.sync.dma_start(out=mask_sb, in_=mask_d[:, :])
        idf_sb = singles.tile([128, 128], F32R, tag="idf")
        nc.sync.dma_start(out=idf_sb, in_=id_d[:, :])
        idb_sb = singles.tile([128, 128], BF16, tag="idb")
        nc.sync.dma_start(out=idb_sb, in_=idb_d[:, :])
        eps_sb = singles.tile([128, 1], F32, tag="eps")
        nc.vector.memset(eps_sb, EPS)

        def layernorm(x_sb, h_sb, st):
            stats = small.tile([128, 6], F32, tag="stats")
            mv = small.tile([128, 2], F32, tag="mv")
            rstd = small.tile([128, 1], F32, tag="rstd")
            nc.vector.bn_stats(out=stats, in_=x_sb)
            nc.vector.bn_aggr(out=mv, in_=stats)
            nc.scalar.activation(out=rstd, in_=mv[:, 1:2],
                                 func=mybir.ActivationFunctionType.Sqrt,
                                 bias=eps_sb, scale=1.0)
            nc.vector.reciprocal(out=rstd, in_=rstd)
            nc.vector.tensor_scalar(out=h_sb, in0=x_sb,
                                    scalar1=mv[:, 0:1], scalar2=rstd,
                                    op0=mybir.AluOpType.subtract,
                                    op1=mybir.AluOpType.mult)

        for it in range(NTILES):
            base = (it % (NTOK // TT)) * TT
            # ---- load x, LN1, transpose h ----
            x_sb = acts.tile([128, NST, C], F32, tag="x")
            h_sb = acts.tile([128, NST, C], F32R, tag="h")
            hT_sb = [acts.tile([128, TT], F32R, tag=f"hT{k}", name=f"hT{k}") for k in range(2)]
            for st in range(NST):
                nc.sync.dma_start(
                    out=x_sb[:, st, :],
                    in_=x_d[base + st * 128: base + (st + 1) * 128, :])
                layernorm(x_sb[:, st, :], h_sb[:, st, :], st)
                for cc in range(2):
                    tp = psB.tile([128, 128], F32, tag="tp")
                    nc.tensor.transpose(out=_r(tp), in_=_r(h_sb[:, st, cc * 128:(cc + 1) * 128]),
                                        identity=_r(idf_sb))
                    nc.vector.tensor_copy(out=hT_sb[cc][:, st * 128:(st + 1) * 128], in_=tp)

            # ---- QKV ----
            qT_sb = [attnp.tile([128, TT], BF16, tag=f"qT{m}", name=f"qT{m}") for m in range(2)]
            kT_sb = [attnp.tile([128, TT], BF16, tag=f"kT{m}", name=f"kT{m}") for m in range(2)]
            for m in range(4):  # 0,1 -> q chunks; 2,3 -> k chunks
                ps = psA.tile([128, TT], F32, tag="psA")
                for k in range(2):
                    nc.tensor.matmul(out=ps,
                                     lhsT=_r(wqkv_sb[k][:, m * 128:(m + 1) * 128]),
                                     rhs=_r(hT_sb[k]),
                                     start=(k == 0), stop=(k == 1))
                dst = qT_sb[m] if m < 2 else kT_sb[m - 2]
                nc.vector.tensor_scalar(out=dst, in0=ps,
                                        scalar1=bqk_sb[:, m:m + 1],
                                        scalar2=CINV if m < 2 else 1.0,
                                        op0=mybir.AluOpType.add,
                                        op1=mybir.AluOpType.mult)
            v_sb = attnp.tile([128, NST, C], BF16, tag="v")
            for st in range(NST):
                ps = psC.tile([128, C], F32, tag="psC")
                for k in range(2):
                    nc.tensor.matmul(out=ps,
                                     lhsT=_r(hT_sb[k][:, st * 128:(st + 1) * 128]),
                                     rhs=_r(wqkv_sb[k][:, 2 * C:3 * C]),
                                     start=(k == 0), stop=(k == 1))
                nc.vector.tensor_add(out=v_sb[:, st, :], in0=ps, in1=bv_b)

            # ---- attention ----
            attnT_sb = [acts.tile([128, TT], F32R, tag=f"aT{hc}", name=f"aT{hc}") for hc in range(2)]
            attnT_ps = [psA.tile([128, TT], F32, tag="psA", name=f"aTps{hc}") for hc in range(2)]
            if SKIP_ATTN:
                for hc in range(2):
                    nc.vector.tensor_copy(out=attnT_sb[hc], in_=hT_sb[hc])
            for st in range(NST if not SKIP_ATTN else 0):
                for h in range(H):
                    hc, off = h // 2, 64 * (h % 2)
                    sl = slice(st * 128, (st + 1) * 128)
                    s_ps = psC.tile([128, 128], F32, tag="psC")
                    nc.tensor.matmul(out=s_ps,
                                     lhsT=qT_sb[hc][off:off + 64, sl],
                                     rhs=kT_sb[hc][off:off + 64, sl],
                                     start=True, stop=True)
                    wn_sb = small.tile([128, 128], BF16, tag="wn")
                    if SKIP_SMAX:
                        nc.vector.tensor_copy(out=wn_sb, in_=s_ps)
                    else:
                        nc.vector.tensor_add(out=s_ps, in0=s_ps, in1=mask_sb)
                        w_sb = small.tile([128, 128], BF16, tag="w")
                        rsum = small.tile([128, 1], F32, tag="rsum")
                        nc.scalar.activation(out=w_sb, in_=s_ps,
                                             func=mybir.ActivationFunctionType.Exp,
                                             accum_out=rsum)
                        rcp = small.tile([128, 1], F32, tag="rcp")
                        nc.vector.reciprocal(out=rcp, in_=rsum)
                        nc.vector.tensor_scalar_mul(out=wn_sb, in0=w_sb, scalar1=rcp)
                    wt_ps = psB.tile([128, 128], BF16, tag="tp")
                    nc.tensor.transpose(out=wt_ps, in_=wn_sb, identity=idb_sb)
                    wt_sb = small.tile([128, 128], BF16, tag="wt")
                    nc.vector.tensor_copy(out=wt_sb, in_=wt_ps)
                    nc.tensor.matmul(out=attnT_ps[hc][off:off + 64, sl],
                                     lhsT=v_sb[:, st, h * 64:(h + 1) * 64],
                                     rhs=wt_sb,
                                     start=True, stop=True)
            for hc in range(2 if not SKIP_ATTN else 0):
                nc.vector.tensor_copy(out=attnT_sb[hc], in_=attnT_ps[hc])

            # ---- Wo + residual, LN2, transpose h2 ----
            x2_sb = acts.tile([128, NST, C], F32, tag="x2")
            h2_sb = acts.tile([128, NST, C], F32R, tag="h2")
            h2T_sb = [acts.tile([128, TT], F32R, tag=f"h2T{k}", name=f"h2T{k}") for k in range(2)]
            for st in range(NST):
                ps = psC.tile([128, C], F32, tag="psC")
                for hc in range(2):
                    nc.tensor.matmul(out=ps,
                                     lhsT=_r(attnT_sb[hc][:, st * 128:(st + 1) * 128]),
                                     rhs=_r(wo_sb[hc]),
                                     start=(hc == 0), stop=(hc == 1))
                nc.vector.scalar_tensor_tensor(out=x2_sb[:, st, :], in0=ps,
                                               scalar=1.0, in1=x_sb[:, st, :],
                                               op0=mybir.AluOpType.mult,
                                               op1=mybir.AluOpType.add)
                if WITH_BIAS:
                    nc.vector.tensor_add(out=x2_sb[:, st, :], in0=x2_sb[:, st, :], in1=bo_b)
                layernorm(x2_sb[:, st, :], h2_sb[:, st, :], st)
                for cc in range(2):
                    tp = psB.tile([128, 128], F32, tag="tp")
                    nc.tensor.transpose(out=_r(tp), in_=_r(h2_sb[:, st, cc * 128:(cc + 1) * 128]),
                                        identity=_r(idf_sb))
                    nc.vector.tensor_copy(out=h2T_sb[cc][:, st * 128:(st + 1) * 128], in_=tp)

            # ---- MLP ----
            m1_sb = [acts.tile([128, TT], F32R, tag=f"m1{mf}", name=f"m1{mf}") for mf in range(8)]
            for mf in range(8 if not SKIP_MLP else 0):
                ps = psA.tile([128, TT], F32, tag="psA")
                for k in range(2):
                    nc.tensor.matmul(out=ps,
                                     lhsT=_r(w1_sb[k][:, mf * 128:(mf + 1) * 128]),
                                     rhs=_r(h2T_sb[k]),
                                     start=(k == 0), stop=(k == 1))
                nc.scalar.activation(out=m1_sb[mf], in_=ps,
                                     func=mybir.ActivationFunctionType.Relu,
                                     bias=b1m_sb[:, mf:mf + 1], scale=1.0)
            o_sb = acts.tile([128, NST, C], F32, tag="o")
            for st in range(NST):
                ps = psC.tile([128, C], F32, tag="psC")
                if SKIP_MLP:
                    nc.vector.memset(ps, 0.0)
                for mf in range(8 if not SKIP_MLP else 0):
                    nc.tensor.matmul(out=ps,
                                     lhsT=_r(m1_sb[mf][:, st * 128:(st + 1) * 128]),
                                     rhs=_r(w2_sb[mf]),
                                     start=(mf == 0), stop=(mf == 7))
                nc.vector.scalar_tensor_tensor(out=o_sb[:, st, :], in0=ps,
                                               scalar=1.0, in1=x2_sb[:, st, :],
                                               op0=mybir.AluOpType.mult,
                                               op1=mybir.AluOpType.add)
                if WITH_BIAS:
                    nc.vector.tensor_add(out=o_sb[:, st, :], in0=o_sb[:, st, :], in1=b2_b)
                nc.sync.dma_start(
                    out=out_d[base + st * 128: base + (st + 1) * 128, :],
                    in_=o_sb[:, st, :])
    nc.finalize()
    return nc


class TileCtx:
    """TileContext wrapper carrying an ExitStack for pools."""

    def __init__(self, nc):
        self._tc = tile.TileContext(nc)
        self._ctx = ExitStack()

    def __enter__(self):
        tc = self._tc.__enter__()
        tc._ctx = self._ctx
        return tc

    def __exit__(self, *a):
        self._ctx.close()
        return self._tc.__exit__(*a)


def _prep_weights(Wq, Wk, Wv, Wo, bo, W1, b1, W2, b2, g1, be1, g2, be2):
    f = np.float32
    Wqf = np.transpose(Wq, (1, 0, 2)).reshape(C, C)  # [c,(h,d)]
    Wkf = np.transpose(Wk, (1, 0, 2)).reshape(C, C)
    Wvf = np.transpose(Wv, (1, 0, 2)).reshape(C, C)
    g1c = g1[:, None]
    wqkv = np.concatenate([g1c * Wqf, g1c * Wkf, g1c * Wvf], axis=1).astype(f)
    bq = be1 @ Wqf
    bk = be1 @ Wkf
    bv = (be1 @ Wvf).astype(f)
    bqk = np.stack([bq[:128], bq[128:], bk[:128], bk[128:]], axis=1).astype(f)
    w1 = (g2[:, None] * W1).astype(f)
    b1e = (b1 + be2 @ W1).astype(f)
    b1m = b1e.reshape(8, 128).T.copy()  # [128, 8]
    # block-diag causal additive mask [128,128]
    m = np.full((128, 128), -1e30, dtype=f)
    for j in range(8):
        blk = np.tril(np.zeros((T, T), dtype=f) + 0.0) * 0.0
        tri = np.triu(np.full((T, T), -1e30, dtype=f), 1)
        m[j * T:(j + 1) * T, j * T:(j + 1) * T] = tri
    return dict(wqkv=wqkv, bqk=bqk, bv=bv, wo=Wo.astype(f), bo=bo.astype(f),
                w1=w1, b1m=b1m.astype(f), w2=W2.astype(f), b2=b2.astype(f),
                maskb=m, ident=np.eye(128, dtype=f),
                identb=np.eye(128).astype(ml_dtypes.bfloat16))


_CACHE = {}


def kernel(x, Wq, Wk, Wv, Wo, bo, W1, b1, W2, b2, g1, be1, g2, be2):
    x = np.asarray(x, dtype=np.float32)
    wts = _prep_weights(np.asarray(Wq), np.asarray(Wk), np.asarray(Wv),
                        np.asarray(Wo), np.asarray(bo), np.asarray(W1),
                        np.asarray(b1), np.asarray(W2), np.asarray(b2),
                        np.asarray(g1), np.asarray(be1), np.asarray(g2),
                        np.asarray(be2))
    global WITH_BIAS
    WITH_BIAS = not (np.all(np.asarray(bo) == 0) and np.all(np.asarray(b2) == 0)
                     and np.all(np.asarray(be1) == 0) and np.all(np.asarray(be2) == 0)
                     and np.all(np.asarray(b1) == 0))
    key = ("nc", WITH_BIAS)
    if key not in _CACHE:
        _CACHE[key] = build_kernel()
    nc = _CACHE[key]
    xs = x.reshape(NCORES, NTOK, C)
    in_maps = [dict(x=np.ascontiguousarray(xs[i]), **wts) for i in range(NCORES)]
    r = run_bass_kernel_spmd(nc, in_maps, list(range(NCORES)))
    _CACHE["last_results"] = r
    res = r.results
    out = np.stack([res[i]["out"] for i in range(NCORES)], axis=0)
    return out.reshape(B, T, C).astype(np.float32)


if __name__ == "__main__":
    nc = build_kernel()
    print("kernel traced OK")

